# revision 28
# baseline (speedup 1.0000x reference)
"""Multi-head causal attention (B=2, T=2048, C=1024, H=16, Dh=64) on 8 TRN2 cores.

Sharding: batch x head tensor-parallel. Core i handles batch i//4 and heads
4*(i%4) .. 4*(i%4)+3. Each core:
  1. loads its batch's x (2048, 1024) and per-core weight slices,
  2. transposes x on the PE, projects qT/kT/vT (feature-on-partition layouts),
  3. runs causal flash attention in scoresT (keys x tokens) layout, softmax
     denominators via a ones-row folded into v', normalization via a K=1
     PE broadcast matmul,
  4. AllGathers yT across the 4 cores of its batch (two token halves,
     overlapped with compute),
  5. computes its 256 output features of the out-projection (+bias).
Host reassembles (concat feature shards, transpose to token-major).

Matmuls run in fp32r (TF32-like, ~1.5e-4 rel err, 4x faster than fp32).
"""

import json

import numpy as np

import concourse.bass as bass
import concourse.mybir as mybir
from concourse.tile import TileContext
from concourse.bass_utils import run_bass_kernel_spmd
from concourse.masks import make_identity, make_upper_triangular

F32 = mybir.dt.float32
F32R = mybir.dt.float32r

N_CORES = 8
B = 2
T = 2048          # tokens per batch (= per core)
C = 1024          # model dim
NH_CORE = 4       # heads per core
DH = 64
FEATS = NH_CORE * DH   # 256 per-core q/k/v/out features
CCH = 512         # attention t-chunk
NCH = T // CCH    # 4 chunks
KTILES = T // 128  # 16 k-tiles
PCH = 512         # projection token chunk
SCALE = 1.0 / 8.0  # 1/sqrt(DH)


def _split_waits_in_bir(bir_bytes: bytes) -> bytes:
    """Workaround: installed walrus rejects >1 sync-wait per instruction."""
    bir = json.loads(bir_bytes)
    changed = False

    def rewrite(insts):
        nonlocal changed
        out = []
        for inst in insts:
            if isinstance(inst, dict):
                for v in inst.values():
                    visit(v)
                si = inst.get("sync_info")
                engine = inst.get("engine")
                if si and engine and len(si.get("on_wait") or []) > 1:
                    waits = si["on_wait"]
                    for i, w in enumerate(waits[:-1]):
                        out.append(
                            {
                                "debug": inst.get("debug", 0),
                                "engine": engine,
                                "ins": [],
                                "name": f"{inst['name']}_ws{i}",
                                "opcode": "EventSemaphore",
                                "outs": [],
                                "sync_info": {"on_update": [], "on_wait": [w]},
                            }
                        )
                    si["on_wait"] = [waits[-1]]
                    changed = True
            out.append(inst)
        insts[:] = out

    def visit(o):
        if isinstance(o, dict):
            for k, v in o.items():
                if k == "instructions" and isinstance(v, list):
                    rewrite(v)
                else:
                    visit(v)
        elif isinstance(o, list):
            for v in o:
                visit(v)

    visit(bir)
    return json.dumps(bir).encode() if changed else bir_bytes


_PATCHED = False


def _apply_walrus_workaround():
    global _PATCHED
    if _PATCHED:
        return
    import concourse.bass_utils as bass_utils
    import concourse.bass2jax as bass2jax

    orig = bass_utils.compile_bir_kernel

    def wrapped(bir_json, tmpdir, neff_name="file.neff"):
        return orig(_split_waits_in_bir(bir_json), tmpdir, neff_name)

    bass_utils.compile_bir_kernel = wrapped
    bass2jax.compile_bir_kernel = wrapped
    _PATCHED = True


def _build_program() -> bass.Bass:
    nc = bass.Bass(num_devices=N_CORES)

    xb = nc.dram_tensor("xb", [T, C], F32, kind="ExternalInput")
    wq = nc.dram_tensor("wq", [FEATS, C], F32, kind="ExternalInput")
    wk = nc.dram_tensor("wk", [FEATS, C], F32, kind="ExternalInput")
    wv = nc.dram_tensor("wv", [FEATS, C], F32, kind="ExternalInput")
    wo = nc.dram_tensor("wo", [FEATS, C], F32, kind="ExternalInput")
    bo = nc.dram_tensor("bo", [FEATS, 1], F32, kind="ExternalInput")
    out = nc.dram_tensor("out", [FEATS, T], F32, kind="ExternalOutput")

    yloc = [nc.dram_tensor(f"yloc{i}", [FEATS, CCH], F32) for i in range(NCH)]
    yfull = [nc.dram_tensor(f"yfull{i}", [C, CCH], F32) for i in range(NCH)]
    groups = [[0, 1, 2, 3], [4, 5, 6, 7]]

    with TileContext(nc) as tc:
        with (
            tc.tile_pool(name="const", bufs=1) as cpool,
            tc.tile_pool(name="wts", bufs=1) as wpool,
            tc.tile_pool(name="wraw", bufs=2) as wraw,
            tc.tile_pool(name="xload", bufs=1) as xload,
            tc.tile_pool(name="xtp", bufs=2) as xtp,
            tc.tile_pool(name="qkv", bufs=1) as qkv,
            tc.tile_pool(name="vchunk", bufs=2) as vchunk,
            tc.tile_pool(name="expw", bufs=3) as expw,
            tc.tile_pool(name="norm", bufs=1) as norm,
            tc.tile_pool(name="oload", bufs=9) as oload,
            tc.tile_pool(name="osb", bufs=2) as osb,
            tc.tile_pool(name="pp", bufs=2, space="PSUM") as pp,
            tc.tile_pool(name="sp", bufs=2, space="PSUM") as sp,
            tc.tile_pool(name="yp", bufs=2, space="PSUM") as yp,
        ):
            # ---- constants (gpsimd iota ops first: keep Pool queue clear) ----
            identity = cpool.tile([128, 128], F32)
            make_identity(nc, identity[:])
            mask = cpool.tile([128, 128], F32)
            make_upper_triangular(nc, mask[:], val=1.0, diag=True)
            mask_r = cpool.tile([128, 128], F32R)
            nc.vector.tensor_copy(out=mask_r[:], in_=mask[:])
            ones_r = cpool.tile([128, 64], F32R)
            nc.vector.memset(ones_r[:].bitcast(F32), 1.0)
            bias_sb = cpool.tile([128, 2], F32)
            nc.sync.dma_start(
                out=bias_sb[:],
                in_=bo.ap().rearrange("(m p) o -> p (m o)", m=2),
            )

            # ---- prefetch x chunk 0 (feeds the first PE work) ----
            xs_pre = []
            for s in range(4):
                xs = xload.tile([128, C], F32, name=f"xsp{s}", tag=f"xs{s}")
                nc.sync.dma_start(out=xs[:], in_=xb[128 * s : 128 * (s + 1), :])
                xs_pre.append(xs)

            # ---- weight transposes: w (feat, C) -> wT[sec][m] (C-chunk, feat) ----
            wT = {}
            for sec, wdram in (("q", wq), ("k", wk), ("v", wv), ("o", wo)):
                for m in range(2):
                    wsb = wraw.tile([128, C], F32, name=f"wsb_{sec}_{m}", tag="wsb")
                    # q/k via SWDGE, v/o via HWDGE: split the load across queues
                    eng = nc.gpsimd if sec in ("q", "k") else nc.sync
                    eng.dma_start(out=wsb[:], in_=wdram[128 * m : 128 * (m + 1), :])
                    wt = wpool.tile([128, 8 * 128], F32R, name=f"wT_{sec}_{m}")
                    for half in range(2):
                        tp = pp.tile([128, 512], F32, name="wtp", tag="pp")
                        for k4 in range(4):
                            k = 4 * half + k4
                            nc.tensor.transpose(
                                tp[:, 128 * k4 : 128 * (k4 + 1)],
                                wsb[:, 128 * k : 128 * (k + 1)],
                                identity[:],
                            )
                        nc.scalar.copy(
                            out=wt[:, 512 * half : 512 * (half + 1)], in_=tp[:]
                        )
                    wT[sec, m] = wt

            # ---- persistent activations ----
            qT = [qkv.tile([128, T], F32R, name=f"qT_{m}") for m in range(2)]
            kT = [qkv.tile([128, T], F32R, name=f"kT_{m}") for m in range(2)]
            vp = {}
            for h in range(NH_CORE):
                for j in range(KTILES):
                    t = qkv.tile([128, DH + 1], F32R, name=f"vp_{h}_{j}")
                    nc.vector.memset(t[:, DH : DH + 1].bitcast(F32), 1.0)
                    vp[h, j] = t

            xt_cache = {}

            def project_chunk(n, secs=("q", "k", "v")):
                """Project tokens [512n, 512n+512): fill qT/kT columns, v' tiles."""
                t0 = PCH * n
                if n in xt_cache:
                    xt = xt_cache.pop(n)
                    skip_xt = True
                else:
                    xt = xtp.tile([128, 8 * PCH], F32R, name="xt", tag="xt")
                    skip_xt = False
                if n == 0:
                    xss = xs_pre
                else:
                    xss = []
                    for s in range(4):
                        xs = xload.tile([128, C], F32, name=f"xs{n}_{s}", tag=f"xs{s}")
                        nc.sync.dma_start(
                            out=xs[:], in_=xb[t0 + 128 * s : t0 + 128 * (s + 1), :]
                        )
                        xss.append(xs)
                if not skip_xt:
                    for k in range(8):
                        tp = pp.tile([128, 512], F32, name="xtr", tag="pp")
                        for s in range(4):
                            nc.tensor.transpose(
                                tp[:, 128 * s : 128 * (s + 1)],
                                xss[s][:, 128 * k : 128 * (k + 1)],
                                identity[:],
                            )
                        nc.vector.tensor_copy(
                            out=xt[:, PCH * k : PCH * (k + 1)], in_=tp[:]
                        )
                if secs != ("q", "k", "v") and "q" in secs:
                    xt_cache[n] = xt  # keep for the k/v pass
                vch = [None, None]
                for sec in secs:
                    for m in range(2):
                        ps = pp.tile([128, PCH], F32, name="projps", tag="pp")
                        for k in range(8):
                            nc.tensor.matmul(
                                ps[:],
                                wT[sec, m][:, 128 * k : 128 * (k + 1)],
                                xt[:, PCH * k : PCH * (k + 1)],
                                start=(k == 0),
                                stop=(k == 7),
                            )
                        cp = nc.scalar.copy if n <= 1 else (
                            lambda out, in_: nc.vector.tensor_copy(out=out, in_=in_)
                        )
                        if sec == "q":
                            cp(out=qT[m][:, t0 : t0 + PCH], in_=ps[:])
                        elif sec == "k":
                            cp(out=kT[m][:, t0 : t0 + PCH], in_=ps[:])
                        else:
                            vc = vchunk.tile([128, PCH], F32, name="vc", tag="vc")
                            cp(out=vc[:], in_=ps[:])
                            vch[m] = vc
                for h in range(NH_CORE if "v" in secs else 0):
                    m, b_ = h // 2, h % 2
                    for jj in range(4):
                        j = 4 * n + jj
                        tp = pp.tile([128, DH], F32, name="vtr", tag="pp")
                        nc.tensor.matmul(
                            tp[:],
                            vch[m][64 * b_ : 64 * (b_ + 1), 128 * jj : 128 * (jj + 1)],
                            identity[64 * b_ : 64 * (b_ + 1), 64 * b_ : 64 * (b_ + 1)],
                            is_transpose=True,
                        )
                        nc.vector.tensor_copy(out=vp[h, j][:, 0:DH], in_=tp[:])

            def attend_part(c, heads, p0, p1, ytps):
                """Attention for tokens [512c, 512c+512), k-tile pairs [p0, p1)."""
                npairs = 2 * c + 2
                jlast = 4 * c + 3
                for h in heads:
                    m, b_ = h // 2, h % 2
                    hq = qT[m][64 * b_ : 64 * (b_ + 1), :]
                    hk = kT[m][64 * b_ : 64 * (b_ + 1), :]
                    if h not in ytps:
                        ytps[h] = yp.tile([DH + 1, CCH], F32, name=f"ytp{c}_{h}", tag="ytp")
                    ytp = ytps[h]
                    for p in range(p0, min(p1, npairs)):
                        sc = sp.tile([128, 1024], F32, name="sc", tag="sc")
                        ex = expw.tile([128, 1024], F32R, name="ex", tag="ex")
                        info = []
                        off = 0
                        for half in range(2):
                            j = 2 * p + half
                            tstart = max(128 * j, CCH * c)
                            w = CCH * (c + 1) - tstart
                            nc.tensor.matmul(
                                sc[0:128, off : off + w],
                                hk[:, 128 * j : 128 * (j + 1)],
                                hq[:, tstart : tstart + w],
                                start=True,
                                stop=True,
                            )
                            info.append((j, tstart, w, off))
                            off += w
                        nc.scalar.activation(
                            ex[:, 0:off],
                            sc[0:128, 0:off],
                            mybir.ActivationFunctionType.Exp,
                            scale=SCALE,
                        )
                        for j, tstart, w, o in info:
                            if 128 * j >= CCH * c:
                                nc.vector.tensor_mul(
                                    out=ex[:, o : o + 128],
                                    in0=ex[:, o : o + 128],
                                    in1=mask_r[:],
                                )
                            lo = tstart - CCH * c
                            nc.tensor.matmul(
                                ytp[0 : DH + 1, lo : lo + w],
                                vp[h, j][:],
                                ex[:, o : o + w],
                                start=(j == 0),
                                stop=(j == jlast),
                            )
                    if min(p1, npairs) == npairs:  # head complete -> normalize
                        den = norm.tile([128, CCH], F32R, name="den", tag="den")
                        nc.vector.tensor_copy(out=den[64:65, :], in_=ytp[DH : DH + 1, :])
                        bc = pp.tile([64, CCH], F32, name="bc", tag="pp")
                        nc.tensor.matmul(
                            bc[:], ones_r[64:65, :], den[64:65, :], start=True, stop=True
                        )
                        bcr = norm.tile([64, CCH], F32, name="bcr", tag="bcr")
                        nc.vector.reciprocal(bcr[:], bc[:])
                        ysb = norm.tile([64, CCH], F32, name="ysb", tag="ysb")
                        nc.vector.tensor_mul(out=ysb[:], in0=ytp[0:DH, :], in1=bcr[:])
                        nc.sync.dma_start(
                            out=yloc[c][DH * h : DH * (h + 1), :], in_=ysb[:]
                        )
                        del ytps[h]

            def attend_chunk(c):
                attend_part(c, range(NH_CORE), 0, 2 * c + 2, {})

            def allgather(i):
                nc.gpsimd.collective_compute(
                    "AllGather",
                    mybir.AluOpType.bypass,
                    replica_groups=groups,
                    ins=[yloc[i][:].opt()],
                    outs=[yfull[i][:].opt()],
                )

            def out_proj(c):
                """Out-projection for token chunk c: out[:, 512c : 512c+512]."""
                yf = []
                for k in range(8):
                    t = oload.tile([128, 512], F32R, name=f"yf{c}_{k}", tag="yf")
                    nc.gpsimd.dma_start(
                        out=t[:], in_=yfull[c][128 * k : 128 * (k + 1), :]
                    )
                    yf.append(t)
                for m in range(2):
                    ps = pp.tile([128, 512], F32, name="ops", tag="pp")
                    for k in range(8):
                        nc.tensor.matmul(
                            ps[:],
                            wT["o", m][:, 128 * k : 128 * (k + 1)],
                            yf[k][:],
                            start=(k == 0),
                            stop=(k == 7),
                        )
                    ob = osb.tile([128, 512], F32, name="ob", tag="ob")
                    nc.vector.tensor_scalar_add(
                        out=ob[:], in0=ps[:], scalar1=bias_sb[:, m : m + 1]
                    )
                    nc.sync.dma_start(
                        out=out[128 * m : 128 * (m + 1), CCH * c : CCH * (c + 1)],
                        in_=ob[:],
                    )

            project_chunk(0)
            attend_chunk(0)
            allgather(0)
            project_chunk(1)
            attend_chunk(1)
            allgather(1)
            out_proj(0)
            project_chunk(2)
            attend_chunk(2)
            allgather(2)
            out_proj(1)
            project_chunk(3, secs=("q",))
            ytps3 = {}
            attend_part(3, [0, 1], 0, 6, ytps3)
            project_chunk(3, secs=("k", "v"))
            attend_part(3, [0, 1], 6, 8, ytps3)
            attend_part(3, [2, 3], 0, 8, ytps3)
            allgather(3)
            out_proj(2)
            out_proj(3)

    return nc


_PROGRAM = None


def _get_program():
    global _PROGRAM
    if _PROGRAM is None:
        _apply_walrus_workaround()
        _PROGRAM = _build_program()
    return _PROGRAM


def kernel(x, w_qkv, w_out, b_out):
    x = np.asarray(x, dtype=np.float32)
    w_qkv = np.asarray(w_qkv, dtype=np.float32)
    w_out = np.asarray(w_out, dtype=np.float32)
    b_out = np.asarray(b_out, dtype=np.float32)

    in_maps = []
    for i in range(N_CORES):
        b, q = divmod(i, 4)
        sl = slice(FEATS * q, FEATS * (q + 1))
        in_maps.append(
            {
                "xb": np.ascontiguousarray(x[b]),
                "wq": np.ascontiguousarray(w_qkv[0 * C :][sl]),
                "wk": np.ascontiguousarray(w_qkv[1 * C :][sl]),
                "wv": np.ascontiguousarray(w_qkv[2 * C :][sl]),
                "wo": np.ascontiguousarray(w_out[sl]),
                "bo": np.ascontiguousarray(b_out[sl].reshape(FEATS, 1)),
            }
        )

    nc = _get_program()
    res = run_bass_kernel_spmd(nc, in_maps, core_ids=list(range(N_CORES)))
    kernel.last_results = res

    outs = []
    for b in range(B):
        big = np.concatenate(
            [res.results[4 * b + q]["out"] for q in range(4)], axis=0
        )
        outs.append(big.T)
    return np.stack(outs).astype(np.float32)


# revision 29
# speedup vs baseline: 1.0094x; 1.0094x over previous
"""Multi-head causal attention (B=2, T=2048, C=1024, H=16, Dh=64) on 8 TRN2 cores.

Sharding: batch x head tensor-parallel. Core i handles batch i//4 and heads
4*(i%4) .. 4*(i%4)+3. Each core:
  1. loads its batch's x (2048, 1024) and per-core weight slices,
  2. transposes x on the PE, projects qT/kT/vT (feature-on-partition layouts),
  3. runs causal flash attention in scoresT (keys x tokens) layout, softmax
     denominators via a ones-row folded into v', normalization via a K=1
     PE broadcast matmul,
  4. AllGathers yT across the 4 cores of its batch (two token halves,
     overlapped with compute),
  5. computes its 256 output features of the out-projection (+bias).
Host reassembles (concat feature shards, transpose to token-major).

Matmuls run in fp32r (TF32-like, ~1.5e-4 rel err, 4x faster than fp32).
"""

import json

import numpy as np

import concourse.bass as bass
import concourse.mybir as mybir
from concourse.tile import TileContext
from concourse.bass_utils import run_bass_kernel_spmd
from concourse.masks import make_identity, make_upper_triangular

F32 = mybir.dt.float32
F32R = mybir.dt.float32r

N_CORES = 8
B = 2
T = 2048          # tokens per batch (= per core)
C = 1024          # model dim
NH_CORE = 4       # heads per core
DH = 64
FEATS = NH_CORE * DH   # 256 per-core q/k/v/out features
CCH = 512         # attention t-chunk
NCH = T // CCH    # 4 chunks
KTILES = T // 128  # 16 k-tiles
PCH = 512         # projection token chunk
SCALE = 1.0 / 8.0  # 1/sqrt(DH)


def _split_waits_in_bir(bir_bytes: bytes) -> bytes:
    """Workaround: installed walrus rejects >1 sync-wait per instruction."""
    bir = json.loads(bir_bytes)
    changed = False

    def rewrite(insts):
        nonlocal changed
        out = []
        for inst in insts:
            if isinstance(inst, dict):
                for v in inst.values():
                    visit(v)
                si = inst.get("sync_info")
                engine = inst.get("engine")
                if si and engine and len(si.get("on_wait") or []) > 1:
                    waits = si["on_wait"]
                    for i, w in enumerate(waits[:-1]):
                        out.append(
                            {
                                "debug": inst.get("debug", 0),
                                "engine": engine,
                                "ins": [],
                                "name": f"{inst['name']}_ws{i}",
                                "opcode": "EventSemaphore",
                                "outs": [],
                                "sync_info": {"on_update": [], "on_wait": [w]},
                            }
                        )
                    si["on_wait"] = [waits[-1]]
                    changed = True
            out.append(inst)
        insts[:] = out

    def visit(o):
        if isinstance(o, dict):
            for k, v in o.items():
                if k == "instructions" and isinstance(v, list):
                    rewrite(v)
                else:
                    visit(v)
        elif isinstance(o, list):
            for v in o:
                visit(v)

    visit(bir)
    return json.dumps(bir).encode() if changed else bir_bytes


_PATCHED = False


def _apply_walrus_workaround():
    global _PATCHED
    if _PATCHED:
        return
    import concourse.bass_utils as bass_utils
    import concourse.bass2jax as bass2jax

    orig = bass_utils.compile_bir_kernel

    def wrapped(bir_json, tmpdir, neff_name="file.neff"):
        return orig(_split_waits_in_bir(bir_json), tmpdir, neff_name)

    bass_utils.compile_bir_kernel = wrapped
    bass2jax.compile_bir_kernel = wrapped
    _PATCHED = True


def _build_program() -> bass.Bass:
    nc = bass.Bass(num_devices=N_CORES)

    xb = nc.dram_tensor("xb", [T, C], F32, kind="ExternalInput")
    wq = nc.dram_tensor("wq", [FEATS, C], F32, kind="ExternalInput")
    wk = nc.dram_tensor("wk", [FEATS, C], F32, kind="ExternalInput")
    wv = nc.dram_tensor("wv", [FEATS, C], F32, kind="ExternalInput")
    wo = nc.dram_tensor("wo", [FEATS, C], F32, kind="ExternalInput")
    bo = nc.dram_tensor("bo", [FEATS, 1], F32, kind="ExternalInput")
    out = nc.dram_tensor("out", [FEATS, T], F32, kind="ExternalOutput")

    yloc = [nc.dram_tensor(f"yloc{i}", [FEATS, CCH], F32) for i in range(NCH)]
    yfull = [nc.dram_tensor(f"yfull{i}", [C, CCH], F32) for i in range(NCH)]
    groups = [[0, 1, 2, 3], [4, 5, 6, 7]]

    with TileContext(nc) as tc:
        with (
            tc.tile_pool(name="const", bufs=1) as cpool,
            tc.tile_pool(name="wts", bufs=1) as wpool,
            tc.tile_pool(name="wraw", bufs=2) as wraw,
            tc.tile_pool(name="xload", bufs=1) as xload,
            tc.tile_pool(name="xtp", bufs=2) as xtp,
            tc.tile_pool(name="qkv", bufs=1) as qkv,
            tc.tile_pool(name="vchunk", bufs=2) as vchunk,
            tc.tile_pool(name="expw", bufs=3) as expw,
            tc.tile_pool(name="norm", bufs=1) as norm,
            tc.tile_pool(name="oload", bufs=9) as oload,
            tc.tile_pool(name="osb", bufs=2) as osb,
            tc.tile_pool(name="pp", bufs=2, space="PSUM") as pp,
            tc.tile_pool(name="sp", bufs=2, space="PSUM") as sp,
            tc.tile_pool(name="yp", bufs=2, space="PSUM") as yp,
        ):
            # ---- constants (gpsimd iota ops first: keep Pool queue clear) ----
            identity = cpool.tile([128, 128], F32)
            make_identity(nc, identity[:])
            mask = cpool.tile([128, 128], F32)
            make_upper_triangular(nc, mask[:], val=1.0, diag=True)
            mask_r = cpool.tile([128, 128], F32R)
            nc.vector.tensor_copy(out=mask_r[:], in_=mask[:])
            ones_r = cpool.tile([128, 64], F32R)
            nc.vector.memset(ones_r[:].bitcast(F32), 1.0)
            bias_sb = cpool.tile([128, 2], F32)
            nc.sync.dma_start(
                out=bias_sb[:],
                in_=bo.ap().rearrange("(m p) o -> p (m o)", m=2),
            )

            # ---- prefetch x chunk 0 (feeds the first PE work) ----
            # column-halved loads: transposes k<4 need only cols [0, 512),
            # so the first 1MB unblocks PE ~3us earlier
            xs_pre = []
            for s in range(4):
                xs = xload.tile([128, C], F32, name=f"xsp{s}", tag=f"xs{s}")
                nc.sync.dma_start(out=xs[:, 0:512], in_=xb[128 * s : 128 * (s + 1), 0:512])
                xs_pre.append(xs)
            for s in range(4):
                nc.sync.dma_start(
                    out=xs_pre[s][:, 512:1024], in_=xb[128 * s : 128 * (s + 1), 512:1024]
                )

            # ---- weight transposes: w (feat, C) -> wT[sec][m] (C-chunk, feat) ----
            wT = {}
            for sec, wdram in (("q", wq), ("k", wk), ("v", wv), ("o", wo)):
                for m in range(2):
                    wsb = wraw.tile([128, C], F32, name=f"wsb_{sec}_{m}", tag="wsb")
                    # q/k via SWDGE, v/o via HWDGE: split the load across queues
                    eng = nc.gpsimd if sec in ("q", "k") else nc.sync
                    eng.dma_start(out=wsb[:], in_=wdram[128 * m : 128 * (m + 1), :])
                    wt = wpool.tile([128, 8 * 128], F32R, name=f"wT_{sec}_{m}")
                    for half in range(2):
                        tp = pp.tile([128, 512], F32, name="wtp", tag="pp")
                        for k4 in range(4):
                            k = 4 * half + k4
                            nc.tensor.transpose(
                                tp[:, 128 * k4 : 128 * (k4 + 1)],
                                wsb[:, 128 * k : 128 * (k + 1)],
                                identity[:],
                            )
                        nc.scalar.copy(
                            out=wt[:, 512 * half : 512 * (half + 1)], in_=tp[:]
                        )
                    wT[sec, m] = wt

            # ---- persistent activations ----
            qT = [qkv.tile([128, T], F32R, name=f"qT_{m}") for m in range(2)]
            kT = [qkv.tile([128, T], F32R, name=f"kT_{m}") for m in range(2)]
            vp = {}
            for h in range(NH_CORE):
                for j in range(KTILES):
                    t = qkv.tile([128, DH + 1], F32R, name=f"vp_{h}_{j}")
                    nc.vector.memset(t[:, DH : DH + 1].bitcast(F32), 1.0)
                    vp[h, j] = t

            xt_cache = {}

            def project_chunk(n, secs=("q", "k", "v")):
                """Project tokens [512n, 512n+512): fill qT/kT columns, v' tiles."""
                t0 = PCH * n
                if n in xt_cache:
                    xt = xt_cache.pop(n)
                    skip_xt = True
                else:
                    xt = xtp.tile([128, 8 * PCH], F32R, name="xt", tag="xt")
                    skip_xt = False
                if n == 0:
                    xss = xs_pre
                else:
                    xss = []
                    for s in range(4):
                        xs = xload.tile([128, C], F32, name=f"xs{n}_{s}", tag=f"xs{s}")
                        nc.sync.dma_start(
                            out=xs[:], in_=xb[t0 + 128 * s : t0 + 128 * (s + 1), :]
                        )
                        xss.append(xs)
                if not skip_xt:
                    for k in range(8):
                        tp = pp.tile([128, 512], F32, name="xtr", tag="pp")
                        for s in range(4):
                            nc.tensor.transpose(
                                tp[:, 128 * s : 128 * (s + 1)],
                                xss[s][:, 128 * k : 128 * (k + 1)],
                                identity[:],
                            )
                        nc.vector.tensor_copy(
                            out=xt[:, PCH * k : PCH * (k + 1)], in_=tp[:]
                        )
                if secs != ("q", "k", "v") and "q" in secs:
                    xt_cache[n] = xt  # keep for the k/v pass
                vch = [None, None]
                for sec in secs:
                    for m in range(2):
                        ps = pp.tile([128, PCH], F32, name="projps", tag="pp")
                        for k in range(8):
                            nc.tensor.matmul(
                                ps[:],
                                wT[sec, m][:, 128 * k : 128 * (k + 1)],
                                xt[:, PCH * k : PCH * (k + 1)],
                                start=(k == 0),
                                stop=(k == 7),
                            )
                        cp = nc.scalar.copy if n <= 1 else (
                            lambda out, in_: nc.vector.tensor_copy(out=out, in_=in_)
                        )
                        if sec == "q":
                            cp(out=qT[m][:, t0 : t0 + PCH], in_=ps[:])
                        elif sec == "k":
                            cp(out=kT[m][:, t0 : t0 + PCH], in_=ps[:])
                        else:
                            vc = vchunk.tile([128, PCH], F32, name="vc", tag="vc")
                            cp(out=vc[:], in_=ps[:])
                            vch[m] = vc
                for h in range(NH_CORE if "v" in secs else 0):
                    m, b_ = h // 2, h % 2
                    for jj in range(4):
                        j = 4 * n + jj
                        tp = pp.tile([128, DH], F32, name="vtr", tag="pp")
                        nc.tensor.matmul(
                            tp[:],
                            vch[m][64 * b_ : 64 * (b_ + 1), 128 * jj : 128 * (jj + 1)],
                            identity[64 * b_ : 64 * (b_ + 1), 64 * b_ : 64 * (b_ + 1)],
                            is_transpose=True,
                        )
                        nc.vector.tensor_copy(out=vp[h, j][:, 0:DH], in_=tp[:])

            def attend_part(c, heads, p0, p1, ytps):
                """Attention for tokens [512c, 512c+512), k-tile pairs [p0, p1)."""
                npairs = 2 * c + 2
                jlast = 4 * c + 3
                for h in heads:
                    m, b_ = h // 2, h % 2
                    hq = qT[m][64 * b_ : 64 * (b_ + 1), :]
                    hk = kT[m][64 * b_ : 64 * (b_ + 1), :]
                    if h not in ytps:
                        ytps[h] = yp.tile([DH + 1, CCH], F32, name=f"ytp{c}_{h}", tag="ytp")
                    ytp = ytps[h]
                    for p in range(p0, min(p1, npairs)):
                        sc = sp.tile([128, 1024], F32, name="sc", tag="sc")
                        ex = expw.tile([128, 1024], F32R, name="ex", tag="ex")
                        info = []
                        off = 0
                        for half in range(2):
                            j = 2 * p + half
                            tstart = max(128 * j, CCH * c)
                            w = CCH * (c + 1) - tstart
                            nc.tensor.matmul(
                                sc[0:128, off : off + w],
                                hk[:, 128 * j : 128 * (j + 1)],
                                hq[:, tstart : tstart + w],
                                start=True,
                                stop=True,
                            )
                            info.append((j, tstart, w, off))
                            off += w
                        nc.scalar.activation(
                            ex[:, 0:off],
                            sc[0:128, 0:off],
                            mybir.ActivationFunctionType.Exp,
                            scale=SCALE,
                        )
                        for j, tstart, w, o in info:
                            if 128 * j >= CCH * c:
                                nc.vector.tensor_mul(
                                    out=ex[:, o : o + 128],
                                    in0=ex[:, o : o + 128],
                                    in1=mask_r[:],
                                )
                            lo = tstart - CCH * c
                            nc.tensor.matmul(
                                ytp[0 : DH + 1, lo : lo + w],
                                vp[h, j][:],
                                ex[:, o : o + w],
                                start=(j == 0),
                                stop=(j == jlast),
                            )
                    if min(p1, npairs) == npairs:  # head complete -> normalize
                        den = norm.tile([128, CCH], F32R, name="den", tag="den")
                        nc.vector.tensor_copy(out=den[64:65, :], in_=ytp[DH : DH + 1, :])
                        bc = pp.tile([64, CCH], F32, name="bc", tag="pp")
                        nc.tensor.matmul(
                            bc[:], ones_r[64:65, :], den[64:65, :], start=True, stop=True
                        )
                        bcr = norm.tile([64, CCH], F32, name="bcr", tag="bcr")
                        nc.vector.reciprocal(bcr[:], bc[:])
                        ysb = norm.tile([64, CCH], F32, name="ysb", tag="ysb")
                        nc.vector.tensor_mul(out=ysb[:], in0=ytp[0:DH, :], in1=bcr[:])
                        nc.sync.dma_start(
                            out=yloc[c][DH * h : DH * (h + 1), :], in_=ysb[:]
                        )
                        del ytps[h]

            def attend_chunk(c):
                attend_part(c, range(NH_CORE), 0, 2 * c + 2, {})

            def allgather(i):
                nc.gpsimd.collective_compute(
                    "AllGather",
                    mybir.AluOpType.bypass,
                    replica_groups=groups,
                    ins=[yloc[i][:].opt()],
                    outs=[yfull[i][:].opt()],
                )

            def out_proj(c):
                """Out-projection for token chunk c: out[:, 512c : 512c+512]."""
                yf = []
                for k in range(8):
                    t = oload.tile([128, 512], F32R, name=f"yf{c}_{k}", tag="yf")
                    nc.gpsimd.dma_start(
                        out=t[:], in_=yfull[c][128 * k : 128 * (k + 1), :]
                    )
                    yf.append(t)
                for m in range(2):
                    ps = pp.tile([128, 512], F32, name="ops", tag="pp")
                    for k in range(8):
                        nc.tensor.matmul(
                            ps[:],
                            wT["o", m][:, 128 * k : 128 * (k + 1)],
                            yf[k][:],
                            start=(k == 0),
                            stop=(k == 7),
                        )
                    ob = osb.tile([128, 512], F32, name="ob", tag="ob")
                    nc.vector.tensor_scalar_add(
                        out=ob[:], in0=ps[:], scalar1=bias_sb[:, m : m + 1]
                    )
                    nc.sync.dma_start(
                        out=out[128 * m : 128 * (m + 1), CCH * c : CCH * (c + 1)],
                        in_=ob[:],
                    )

            project_chunk(0)
            attend_chunk(0)
            allgather(0)
            project_chunk(1)
            attend_chunk(1)
            allgather(1)
            out_proj(0)
            project_chunk(2)
            attend_chunk(2)
            allgather(2)
            out_proj(1)
            project_chunk(3, secs=("q",))
            ytps3 = {}
            attend_part(3, [0, 1], 0, 6, ytps3)
            project_chunk(3, secs=("k", "v"))
            attend_part(3, [0, 1], 6, 8, ytps3)
            attend_part(3, [2, 3], 0, 8, ytps3)
            allgather(3)
            out_proj(2)
            out_proj(3)

    return nc


_PROGRAM = None


def _get_program():
    global _PROGRAM
    if _PROGRAM is None:
        _apply_walrus_workaround()
        _PROGRAM = _build_program()
    return _PROGRAM


def kernel(x, w_qkv, w_out, b_out):
    x = np.asarray(x, dtype=np.float32)
    w_qkv = np.asarray(w_qkv, dtype=np.float32)
    w_out = np.asarray(w_out, dtype=np.float32)
    b_out = np.asarray(b_out, dtype=np.float32)

    in_maps = []
    for i in range(N_CORES):
        b, q = divmod(i, 4)
        sl = slice(FEATS * q, FEATS * (q + 1))
        in_maps.append(
            {
                "xb": np.ascontiguousarray(x[b]),
                "wq": np.ascontiguousarray(w_qkv[0 * C :][sl]),
                "wk": np.ascontiguousarray(w_qkv[1 * C :][sl]),
                "wv": np.ascontiguousarray(w_qkv[2 * C :][sl]),
                "wo": np.ascontiguousarray(w_out[sl]),
                "bo": np.ascontiguousarray(b_out[sl].reshape(FEATS, 1)),
            }
        )

    nc = _get_program()
    res = run_bass_kernel_spmd(nc, in_maps, core_ids=list(range(N_CORES)))
    kernel.last_results = res

    outs = []
    for b in range(B):
        big = np.concatenate(
            [res.results[4 * b + q]["out"] for q in range(4)], axis=0
        )
        outs.append(big.T)
    return np.stack(outs).astype(np.float32)
